# revision 1
# baseline (speedup 1.0000x reference)
"""Causal multi-head self-attention on 8 TRN2 NeuronCores.

Sharding: core = (batch b, head-group g): 4 batches x 2 groups of 8 heads.
Host pre-transposes all operands so every TensorE matmul contracts over the
partition dim with zero on-device transposes:

  phase 1a: qk^T[n, i]  = sum_k Wqk[n, k] xT[k, i]      (lhsT=WqkT blk, rhs=xT)
  phase 1b: v[j, n]     = sum_k xT[k, j] WvT[k, n]      (lhsT=xT blk,   rhs=WvT)
  phase 2 (per head, per 512-wide i-chunk, per 128-deep j-block):
            S^T[j, i]   = sum_d kT[d, j] qT[d, i]       (lhsT=kT blk,   rhs=qT)
            A^T         = exp(S^T / 8) * causal_mask    (ACT + DVE)
            Yaug^T[n,i] = sum_j v_aug[j, n] A^T[j, i]   (lhsT=v_aug,    rhs=A^T)
              where v_aug has a ones column: row 64 of Yaug^T = softmax denom l
            y^T         = Yaug^T[0:64] * (1/l)          (recip + partition bcast)
  phase 3:  out[i, o]   = sum_n yT[n, i] WpT[n, o]      (lhsT=yT blk,   rhs=WpT)

All matmul operands are float32r (TF32-like, 1 cycle/row at N>=256, ~1.5e-4
matmul rel err); PSUM accumulation is fp32.  Softmax skips max-subtraction
(scores are O(+-10), exp is safe in fp32) so the denominator comes free from
the ones-column trick.  The two per-batch head-group partials are summed on
the host at gather time.
"""

import numpy as np

import concourse.mybir as mybir
import concourse.tile as tile
from concourse import bacc
from concourse.bass_utils import run_bass_kernel_spmd

F32 = mybir.dt.float32
F32R = mybir.dt.float32r
BF16 = mybir.dt.bfloat16
Exp = mybir.ActivationFunctionType.Exp

COMPUTE = "f32r"          # "f32r" | "bf16"


def _cdt():
    return F32R if COMPUTE == "f32r" else BF16


def set_compute(name):
    global COMPUTE
    assert name in ("f32r", "bf16")
    COMPUTE = name
    _CACHE.clear()

B, C, H = 4, 1024, 16
HPC = 8            # heads per core
HD = 64            # head dim
GQ = HPC * HD      # 512 columns per head group
P = 128
KB = C // P        # 8 k-blocks
SCALE = 0.125      # 1/sqrt(HD)


def build(T=2048, ps1_bufs=2, psS_bufs=2, psY_bufs=2, at_bufs=8, dup=1, ic=512, flat=True):
    CDT = _cdt()
    nT = T // P      # j-blocks
    nI = T // 512    # i-chunks
    nc = bacc.Bacc("TRN2", target_bir_lowering=False, debug=False, num_devices=8)

    xT = nc.dram_tensor("xT", [C, T], CDT, kind="ExternalInput").ap()
    wqkT = nc.dram_tensor("wqkT", [C, 2 * GQ], CDT, kind="ExternalInput").ap()
    wvT = nc.dram_tensor("wvT", [C, GQ], CDT, kind="ExternalInput").ap()
    wpT = nc.dram_tensor("wpT", [GQ, C], CDT, kind="ExternalInput").ap()
    maskT = nc.dram_tensor("maskT", [P, 2 * P], CDT, kind="ExternalInput").ap()
    onesT = nc.dram_tensor("onesT", [P, (T // P) * HPC], CDT, kind="ExternalInput").ap()
    out = nc.dram_tensor("out", [T, C], F32, kind="ExternalOutput").ap()

    from contextlib import ExitStack
    with tile.TileContext(nc) as tc:
      for _rep in range(dup):
        with tc.tile_pool(name="persist", bufs=1) as pe, ExitStack() as stk:
            pools = None
            if flat:
                pools = (
                    stk.enter_context(tc.tile_pool(name="psSf", bufs=psS_bufs, space="PSUM")),
                    stk.enter_context(tc.tile_pool(name="psYf", bufs=psY_bufs, space="PSUM")),
                    stk.enter_context(tc.tile_pool(name="ps3f", bufs=ps1_bufs, space="PSUM")),
                )

            qk_sb = pe.tile([P, 8 * T], CDT, tag="qk")      # n-blocks 0-3 q, 4-7 k
            v_sb = pe.tile([P, nT * HPC * (HD + 1)], CDT, tag="v")
            mask_sb = pe.tile([P, 2 * P], CDT, tag="mask")
            nc.sync.dma_start(mask_sb[:], maskT)
            nc.sync.dma_start(
                v_sb[:].rearrange("p (j h w) -> p j h w", j=nT, h=HPC)[:, :, :, HD:HD + 1],
                onesT.rearrange("p (j h) -> p j h", j=nT)[:, :, :, None])

            with tc.tile_pool(name="ph1", bufs=1) as p1, ExitStack() as stk1:
                ps1 = pools[2] if pools else stk1.enter_context(
                    tc.tile_pool(name="ps1", bufs=ps1_bufs, space="PSUM"))
                x_sb = p1.tile([P, KB * T], CDT, tag="x")
                wv_sb = p1.tile([P, KB * GQ], CDT, tag="wv")

                def qk_half(w_sb, half, mc_major):
                    order = ([(nb, mc) for mc in range(nI) for nb in range(4)]
                             if mc_major else
                             [(nb, mc) for nb in range(4) for mc in range(nI)])
                    for nb, mc in order:
                        pt = ps1.tile([P, 512], F32, tag="ps1")
                        for kb in range(KB):
                            nc.tensor.matmul(
                                pt[:],
                                w_sb[:, kb * GQ + nb * P: kb * GQ + (nb + 1) * P],
                                x_sb[:, kb * T + mc * 512: kb * T + (mc + 1) * 512],
                                start=(kb == 0), stop=(kb == KB - 1))
                        nc.vector.tensor_copy(
                            qk_sb[:, (4 * half + nb) * T + mc * 512:
                                  (4 * half + nb) * T + (mc + 1) * 512], pt[:])

                # ---- phase 1a-q first: wq + mc-chunked x DMAs pipeline the startup ----
                with tc.tile_pool(name="wqk0", bufs=1) as pw:
                    w_sb = pw.tile([P, KB * GQ], CDT, tag="w0")
                    for kb in range(KB):
                        nc.sync.dma_start(
                            w_sb[:, kb * GQ:(kb + 1) * GQ],
                            wqkT[kb * P:(kb + 1) * P, 0:GQ])
                    for mc in range(nI):
                        for kb in range(KB):
                            nc.sync.dma_start(
                                x_sb[:, kb * T + mc * 512: kb * T + (mc + 1) * 512],
                                xT[kb * P:(kb + 1) * P, mc * 512:(mc + 1) * 512])
                    qk_half(w_sb, 0, mc_major=True)

                # ---- phase 1b: v = x @ Wv^T (x now resident) ----
                for kb in range(KB):
                    nc.sync.dma_start(wv_sb[:, kb * GQ:(kb + 1) * GQ], wvT[kb * P:(kb + 1) * P, :])
                for jb in range(nT):
                    pt = ps1.tile([P, GQ], F32, tag="ps1")
                    for kb in range(KB):
                        nc.tensor.matmul(
                            pt[:],
                            x_sb[:, kb * T + jb * P: kb * T + (jb + 1) * P],
                            wv_sb[:, kb * GQ:(kb + 1) * GQ],
                            start=(kb == 0), stop=(kb == KB - 1))
                    vv = v_sb[:, jb * HPC * (HD + 1):(jb + 1) * HPC * (HD + 1)] \
                        .rearrange("p (h w) -> p h w", h=HPC)
                    nc.vector.tensor_copy(vv[:, :, 0:HD], pt[:].rearrange("p (h w) -> p h w", h=HPC))

                # ---- phase 1a-k ----
                with tc.tile_pool(name="wqk1", bufs=1) as pw:
                    w_sb = pw.tile([P, KB * GQ], CDT, tag="w1")
                    for kb in range(KB):
                        nc.sync.dma_start(
                            w_sb[:, kb * GQ:(kb + 1) * GQ],
                            wqkT[kb * P:(kb + 1) * P, GQ:2 * GQ])
                    qk_half(w_sb, 1, mc_major=False)

            # ---- phases 2+3 ----
            with tc.tile_pool(name="p23", bufs=1) as p23, \
                 tc.tile_pool(name="wrk", bufs=at_bufs) as wrk, \
                 tc.tile_pool(name="fin", bufs=2) as fin:
                yt_sb = p23.tile([P, 4 * T], CDT, tag="yt")
                wp_sb = p23.tile([P, 4 * C], CDT, tag="wp")
                phase23(nc, tc, T, nT, out, qk_sb, v_sb, mask_sb,
                        yt_sb, wp_sb, wpT, wrk, fin, ic, psS_bufs, psY_bufs, pools)
    return nc


def phase23(nc, tc, T, nT, out, qk_sb, v_sb, mask_sb, yt_sb, wp_sb, wpT,
            wrk, fin, ic, psS_bufs, psY_bufs, pools=None):
    """ci-outer flash attention with paired-exp full blocks + interleaved proj.

    Full (non-diagonal-crossing) j-blocks are processed in pairs sharing one
    [128, 1024] PSUM tile so a single ACT exp covers both (ACT instruction
    overhead is the phase-2 bottleneck).  After all heads finish an i-chunk,
    that chunk's 4 proj m-blocks run, spreading output DMA across phase 2.
    """
    CDT = _cdt()
    assert ic == 512
    nCh = T // ic      # i-chunks
    cpb = ic // P      # j-blocks per i-chunk span (4)
    for kb in range(4):
        nc.sync.dma_start(wp_sb[:, kb * C:(kb + 1) * C], wpT[kb * P:(kb + 1) * P, :])

    from contextlib import ExitStack
    with ExitStack() as stk:
        if pools:
            psS, psY, ps3 = pools
        else:
            psS = stk.enter_context(tc.tile_pool(name="psS", bufs=psS_bufs, space="PSUM"))
            psY = stk.enter_context(tc.tile_pool(name="psY", bufs=psY_bufs, space="PSUM"))
            ps3 = stk.enter_context(tc.tile_pool(name="ps3", bufs=2, space="PSUM"))

        for ci in range(nCh):
            jfull = cpb * ci               # full j-blocks (a=0), always even
            jmax = jfull + cpb
            for hp in range(0, HPC, 2):    # head PAIRS interleaved to hide exp latency
                hs = (hp, hp + 1)
                st = {}
                for h in hs:
                    st[h] = dict(
                        po=(h % 2) * HD, qc=(h // 2) * T, kc=(4 + h // 2) * T,
                        vc=h * (HD + 1),
                        py=psY.tile([HD + 1, 512], F32, tag="psY", name=f"py{ci}_{h}"))

                def st_mm(h, dst, jb, a):
                    s = st[h]
                    nc.tensor.matmul(
                        dst,
                        qk_sb[s["po"]:s["po"] + HD, s["kc"] + jb * P: s["kc"] + (jb + 1) * P],
                        qk_sb[s["po"]:s["po"] + HD,
                              s["qc"] + ci * 512 + a: s["qc"] + (ci + 1) * 512],
                        start=True, stop=True)

                def av_mm(h, jb, at_ap, a):
                    s = st[h]
                    nc.tensor.matmul(
                        s["py"][:, a:512],
                        v_sb[:, jb * HPC * (HD + 1) + s["vc"]:
                             jb * HPC * (HD + 1) + s["vc"] + HD + 1],
                        at_ap,
                        start=(jb == 0), stop=(jb == jmax - 1))

                for j0 in range(0, jfull, 2):          # paired full blocks, 2 heads zipped
                    ats = {}
                    for h in hs:
                        psp = psS.tile([P, 1024], F32, tag="psS", name=f"psp{ci}_{h}_{j0}")
                        st_mm(h, psp[:, 0:512], j0, 0)
                        st_mm(h, psp[:, 512:1024], j0 + 1, 0)
                        at = wrk.tile([P, 1024], CDT, tag="at", name=f"at{ci}_{h}_{j0}")
                        nc.scalar.activation(at[:], psp[:], Exp, scale=SCALE)
                        ats[h] = at
                    for h in hs:
                        av_mm(h, j0, ats[h][:, 0:512], 0)
                        av_mm(h, j0 + 1, ats[h][:, 512:1024], 0)

                for p_ in range(cpb):                  # crossing blocks: 2 heads packed
                    jb = jfull + p_
                    a = min(128 * p_, ic - 256)
                    w = 512 - a
                    mw = 128 * p_ - a + P              # 128, or 256 when clamped
                    psp = psS.tile([P, 1024], F32, tag="psS", name=f"psx{ci}_{hp}_{p_}")
                    st_mm(hs[0], psp[:, a:512], jb, a)
                    st_mm(hs[1], psp[:, 512:512 + w], jb, a)   # packed right after
                    at = wrk.tile([P, 1024], CDT, tag="at", name=f"atx{ci}_{hp}_{p_}")
                    nc.scalar.activation(at[:, a:512 + w], psp[:, a:512 + w], Exp, scale=SCALE)
                    nc.vector.tensor_mul(
                        at[:, a:a + mw], at[:, a:a + mw], mask_sb[:, 2 * P - mw:2 * P])
                    nc.vector.tensor_mul(
                        at[:, 512:512 + mw], at[:, 512:512 + mw], mask_sb[:, 2 * P - mw:2 * P])
                    av_mm(hs[0], jb, at[:, a:512], a)
                    av_mm(hs[1], jb, at[:, 512:512 + w], a)

                for h in hs:
                    s = st[h]
                    rt = fin.tile([1, 512], F32, tag="rt")
                    nc.vector.reciprocal(rt[:], s["py"][HD:HD + 1, :])
                    rb = fin.tile([HD, 512], F32, tag="rb")
                    nc.gpsimd.partition_broadcast(rb[:], rt[:])
                    nc.vector.tensor_mul(
                        yt_sb[s["po"]:s["po"] + HD,
                              s["qc"] + ci * 512: s["qc"] + (ci + 1) * 512],
                        s["py"][0:HD, :], rb[:])

            # ---- interleaved proj for this i-chunk's m-blocks ----
            for mb in range(cpb * ci, cpb * (ci + 1)):
                for oc in range(2):
                    po_ = ps3.tile([P, 512], F32, tag="ps1")
                    for nb in range(4):
                        nc.tensor.matmul(
                            po_[:],
                            yt_sb[:, nb * T + mb * P: nb * T + (mb + 1) * P],
                            wp_sb[:, nb * C + oc * 512: nb * C + (oc + 1) * 512],
                            start=(nb == 0), stop=(nb == 3))
                    ot = wrk.tile([P, 512], F32, tag="ot", bufs=3)
                    nc.vector.tensor_copy(ot[:], po_[:])
                    nc.sync.dma_start(out[mb * P:(mb + 1) * P, oc * 512:(oc + 1) * 512], ot[:])


_CACHE = {}


def get_nc(T=2048):
    if T not in _CACHE:
        nc = build(T)
        nc.compile()
        _CACHE[T] = nc
    return _CACHE[T]


def make_in_maps(x, W_attn, W_proj):
    Bx, T, Cx = x.shape
    Wq, Wk, Wv = W_attn[:Cx], W_attn[Cx:2 * Cx], W_attn[2 * Cx:]
    import ml_dtypes
    cv = (lambda a: np.ascontiguousarray(a)) if COMPUTE == "f32r" else (
        lambda a: np.ascontiguousarray(a).astype(ml_dtypes.bfloat16))
    r = np.arange(P)
    tri = (r[:, None] <= r[None, :]).astype(np.float32)
    mask = np.concatenate([np.zeros((P, P), np.float32), tri], axis=1)
    ones = np.ones((P, (T // P) * HPC), np.float32)
    in_maps = []
    for core in range(8):
        b, g = divmod(core, 2)
        rows = slice(g * GQ, (g + 1) * GQ)
        in_maps.append({
            "xT": cv(x[b].T),
            "wqkT": cv(np.concatenate([Wq[rows], Wk[rows]], 0).T),
            "wvT": cv(Wv[rows].T),
            "wpT": cv(W_proj[:, rows].T),
            "maskT": cv(mask),
            "onesT": cv(ones),
        })
    return in_maps


def kernel(x, W_attn, W_proj):
    x = np.asarray(x, dtype=np.float32)
    W_attn = np.asarray(W_attn, dtype=np.float32)
    W_proj = np.asarray(W_proj, dtype=np.float32)
    Bx, T, Cx = x.shape
    assert (Bx, Cx) == (B, C) and W_attn.shape == (3 * C, C) and W_proj.shape == (C, C)
    nc = get_nc(T)
    res = run_bass_kernel_spmd(nc, make_in_maps(x, W_attn, W_proj), list(range(8)))
    out = np.empty((Bx, T, Cx), np.float32)
    for b in range(Bx):
        out[b] = res.results[2 * b]["out"] + res.results[2 * b + 1]["out"]
    return out


if __name__ == "__main__":
    rng = np.random.default_rng(0)
    x = rng.standard_normal((B, 2048, C), dtype=np.float32)
    W_attn = rng.standard_normal((3 * C, C), dtype=np.float32) * (1.0 / np.sqrt(C))
    W_proj = rng.standard_normal((C, C), dtype=np.float32) * (1.0 / np.sqrt(C))
    out = kernel(x, W_attn, W_proj)
    print("out", out.shape, out.dtype, np.abs(out).max())



# revision 15
# speedup vs baseline: 2.8250x; 2.8250x over previous
"""Causal multi-head self-attention on 8 TRN2 NeuronCores.

Sharding: core = (batch b, head-group g): 4 batches x 2 groups of 8 heads.
Host pre-transposes all operands so every TensorE matmul contracts over the
partition dim with zero on-device transposes:

  qkv:      qk^T[n, i]  = sum_k Wqk[n, k] xT[k, i]      (lhsT=WqkT blk, rhs=xT)
            v[j, n]     = sum_k xT[k, j] WvT[k, n]      (lhsT=xT blk,   rhs=WvT)
  attn (per head, per 512-wide i-chunk, per 128-deep j-block):
            S^T[j, i]   = sum_d kT[d, j] qT[d, i]       (lhsT=kT blk,   rhs=qT)
            A^T         = exp(S^T / 8) * causal_mask    (ACT + DVE), bf16
            Yaug^T[n,i] = sum_j v_aug[j, n] A^T[j, i]   (lhsT=v_aug,    rhs=A^T)
              where v_aug has a ones column: row 64 of Yaug^T = softmax denom l
            y^T         = Yaug^T[0:64] * (1/l)          (recip + partition bcast)
  proj:     out[i, o]   = sum_n yT[n, i] WpT[n, o]      (lhsT=yT blk,   rhs=WpT)

v2: single fused pipeline.  The qkv matmuls (pure TensorE) are chopped into
256-column "rounds" and fed into the ACT-bound attention loop by a
deficit-driven feeder, so the scalar engine (exp) and tensor engine stay
concurrently busy instead of running in serial phases.  Each i-chunk's
projection is likewise drained into the next chunk's attention.  Heads are
processed in pairs on partition halves 0-63/64-127 (auto row-tiling packs the
two K=64 S^T matmuls onto disjoint PE row-groups); the pair shares one
[65, 1024] PSUM Y tile so reciprocal/broadcast run once per pair.  exp output
is bf16 (matmul moving operand; ~4e-3 elementwise, averages out in AV).
"""

import numpy as np

import concourse.mybir as mybir
import concourse.tile as tile
from concourse import bacc
from concourse.bass_utils import run_bass_kernel_spmd

F32 = mybir.dt.float32
F32R = mybir.dt.float32r
BF16 = mybir.dt.bfloat16
Exp = mybir.ActivationFunctionType.Exp

B, C, H = 4, 1024, 16
HPC = 8            # heads per core
HD = 64            # head dim
GQ = HPC * HD      # 512 columns per head group
P = 128
KB = C // P        # 8 k-blocks
SCALE = 0.125      # 1/sqrt(HD)
XW = 256           # x round width (columns of T per feeder round)

# dtypes must match within each matmul's (lhsT, rhs) pair: walrus rejects
# mixing 32-bit (f32/f32r) with 16-bit inputs.  Pairs: qkv GEMM (x,w),
# S^T (k,q), AV (v,at), proj (yt,wp).
AGGR = True        # True: bf16 x/w/v/at/yt (S^T stays f32r).  False: bf16 only v/at.
XDT = BF16 if AGGR else F32R   # x, and qkv weights (same GEMM pair)
WDT = XDT
QKDT = F32R        # q, k (S^T pair) - exp argument stays near-exact
VDT = BF16         # v, at (AV pair)
ADT = BF16         # exp output + mask
YDT = BF16 if AGGR else F32R   # yt, wp (proj pair)

# serial cost-model estimates (ns) used only for feeder pacing
_MM = 0.4167       # ns per streamed matmul column @2.4GHz
_ACT = 1.0 / 1.2   # ns per element-column on ACT
_ACTF = 172 / 1.2 + 57  # fixed per-ACT overhead


def build(T=2048, dup=1, at_bufs=8, ps1_bufs=2, psS_bufs=2, psY_bufs=2):
    nT = T // P        # j-blocks (16)
    nCh = T // 512     # i-chunks (4)
    nX = T // XW       # x rounds (8)
    rpc = 512 // XW    # rounds per i-chunk (2)
    nc = bacc.Bacc("TRN2", target_bir_lowering=False, debug=False, num_devices=8)

    xT = nc.dram_tensor("xT", [C, T], XDT, kind="ExternalInput").ap()
    wqkT = nc.dram_tensor("wqkT", [C, 2 * GQ], WDT, kind="ExternalInput").ap()
    wvT = nc.dram_tensor("wvT", [C, GQ], WDT, kind="ExternalInput").ap()
    wpT = nc.dram_tensor("wpT", [GQ, C], YDT, kind="ExternalInput").ap()
    maskT = nc.dram_tensor("maskT", [P, 2 * P], ADT, kind="ExternalInput").ap()
    onesT = nc.dram_tensor("onesT", [P, nT * HPC], VDT, kind="ExternalInput").ap()
    out = nc.dram_tensor("out", [T, C], F32, kind="ExternalOutput").ap()

    with tile.TileContext(nc) as tc:
      for _rep in range(dup):
        with tc.tile_pool(name="persist", bufs=1) as pe, \
             tc.tile_pool(name="roll", bufs=2) as roll, \
             tc.tile_pool(name="wrk", bufs=at_bufs) as wrk, \
             tc.tile_pool(name="fin", bufs=1) as fin, \
             tc.tile_pool(name="psS", bufs=psS_bufs, space="PSUM") as psS, \
             tc.tile_pool(name="psY", bufs=psY_bufs, space="PSUM") as psY, \
             tc.tile_pool(name="ps1", bufs=ps1_bufs, space="PSUM") as ps1:

            k_sb = pe.tile([P, 4 * T], QKDT, tag="k")   # 4 nb-blocks (2 heads each)
            v_sb = pe.tile([P, nT * HPC * (HD + 1)], VDT, tag="v")
            wq_sb = pe.tile([P, KB * GQ], WDT, tag="wq")
            wk_sb = pe.tile([P, KB * GQ], WDT, tag="wk")
            wv_sb = pe.tile([P, KB * GQ], WDT, tag="wv")
            wp_sb = pe.tile([P, 4 * C], YDT, tag="wp")
            mask_sb = pe.tile([P, 2 * P], ADT, tag="mask")

            # rolling chunked tiles: x rounds, q chunks, y chunks
            x_t = [roll.tile([P, KB * XW], XDT, tag="x", name=f"x{r}", bufs=4)
                   for r in range(nX)]
            q_t = [roll.tile([P, 4 * 512], QKDT, tag="q", name=f"q{ci}")
                   for ci in range(nCh)]
            yt_t = [roll.tile([P, 4 * 512], YDT, tag="yt", name=f"yt{ci}")
                    for ci in range(nCh)]

            def xdma(r):
                for kb in range(KB):
                    nc.sync.dma_start(
                        x_t[r][:, kb * XW:(kb + 1) * XW],
                        xT[kb * P:(kb + 1) * P, r * XW:(r + 1) * XW])

            # ---- prologue DMAs (ordered by first consumer) ----
            nc.sync.dma_start(mask_sb[:], maskT)
            scr = fin.tile([P, 2], F32, tag="scr")
            nc.scalar.activation(scr[:], mask_sb[:, 0:2], Exp)  # preload act table
            for kb in range(KB):
                nc.sync.dma_start(wq_sb[:, kb * GQ:(kb + 1) * GQ],
                                  wqkT[kb * P:(kb + 1) * P, 0:GQ])
                nc.sync.dma_start(x_t[0][:, kb * XW:(kb + 1) * XW],
                                  xT[kb * P:(kb + 1) * P, 0:XW])
            for kb in range(KB):
                nc.sync.dma_start(wk_sb[:, kb * GQ:(kb + 1) * GQ],
                                  wqkT[kb * P:(kb + 1) * P, GQ:2 * GQ])
                nc.sync.dma_start(x_t[1][:, kb * XW:(kb + 1) * XW],
                                  xT[kb * P:(kb + 1) * P, XW:2 * XW])
            nc.sync.dma_start(
                v_sb[:].rearrange("p (j h w) -> p j h w", j=nT, h=HPC)[:, :, :, HD:HD + 1],
                onesT.rearrange("p (j h) -> p j h", j=nT)[:, :, :, None])
            for kb in range(KB):
                nc.sync.dma_start(wv_sb[:, kb * GQ:(kb + 1) * GQ],
                                  wvT[kb * P:(kb + 1) * P, :])
            xdma(2)
            xdma(3)
            for kb in range(4):
                nc.sync.dma_start(wp_sb[:, kb * C:(kb + 1) * C],
                                  wpT[kb * P:(kb + 1) * P, :])

            # ---- phase-1 units ----
            def qk_unit(r, nb, w_sb, dst, dcol):
                # dst[:, dcol:dcol+XW] = (w block nb).T @ x round r
                def emit():
                    pt = ps1.tile([P, XW], F32, tag="ps1", name=f"p1_{r}_{nb}")
                    for kb in range(KB):
                        nc.tensor.matmul(
                            pt[:],
                            w_sb[:, kb * GQ + nb * P: kb * GQ + (nb + 1) * P],
                            x_t[r][:, kb * XW:(kb + 1) * XW],
                            start=(kb == 0), stop=(kb == KB - 1))
                    nc.vector.tensor_copy(dst[:, dcol:dcol + XW], pt[:])
                return emit, KB * XW * _MM

            def v_unit(r, jj):
                jb = (r * XW) // P + jj
                def emit():
                    pt = ps1.tile([P, GQ], F32, tag="ps1", name=f"pv_{r}_{jj}")
                    for kb in range(KB):
                        nc.tensor.matmul(
                            pt[:],
                            x_t[r][:, kb * XW + jj * P: kb * XW + (jj + 1) * P],
                            wv_sb[:, kb * GQ:(kb + 1) * GQ],
                            start=(kb == 0), stop=(kb == KB - 1))
                    vv = v_sb[:, jb * HPC * (HD + 1):(jb + 1) * HPC * (HD + 1)] \
                        .rearrange("p (h w) -> p h w", h=HPC)
                    nc.vector.tensor_copy(vv[:, :, 0:HD],
                                          pt[:].rearrange("p (h w) -> p h w", h=HPC))
                return emit, KB * GQ * _MM

            # feeder: [(deadline, emit, pe_ns), ...] sorted by deadline.
            # deadline d means: must be emitted before attention pair
            # (ci=floor(d), hp=2*int(4*frac(d))) starts.
            feeder = []
            for c in range(nCh):
                r0, r1 = 2 * c, 2 * c + 1
                for nb in range(4):
                    for r in (r0, r1):
                        off = (r * XW) % 512
                        feeder.append(
                            (c + nb / 4,)
                            + qk_unit(r, nb, wq_sb, q_t[c], nb * 512 + off))
                        feeder.append(
                            (c + nb / 4,)
                            + qk_unit(r, nb, wk_sb, k_sb, nb * T + r * XW))
                    if nb == 0:  # v due at pair 0's av, just after nb0 q/k
                        for r in (r0, r1):
                            for jj in range(XW // P):
                                feeder.append((float(c),) + v_unit(r, jj))
                for r in (r0, r1):
                    if r + 4 < nX:
                        feeder.append(
                            (c + 0.8 + (r % 2) / 20, (lambda rr=r + 4: xdma(rr)), 0.0))
            fpos = [0]

            def drain_feeder(max_d, deficit=None):
                # deficit None -> force-drain everything with deadline <= max_d
                d = deficit
                while fpos[0] < len(feeder):
                    dl, emit, cost = feeder[fpos[0]]
                    if dl > max_d:
                        break
                    if d is not None:
                        if d < cost:
                            break
                        d -= cost
                    emit()
                    fpos[0] += 1
                return 0.0 if d is None else d

            # ---- proj units (chunk ci) ----
            def proj_unit(ci, mb):
                def emit():
                    for oc in range(2):
                        po_ = ps1.tile([P, 512], F32, tag="ps1", name=f"po{mb}_{oc}")
                        for nb in range(4):
                            nc.tensor.matmul(
                                po_[:],
                                yt_t[ci][:, nb * 512 + (mb - 4 * ci) * P:
                                         nb * 512 + (mb - 4 * ci + 1) * P],
                                wp_sb[:, nb * C + oc * 512: nb * C + (oc + 1) * 512],
                                start=(nb == 0), stop=(nb == 3))
                        ot = wrk.tile([P, 512], F32, tag="ot", bufs=3, name=f"ot{mb}_{oc}")
                        nc.vector.tensor_copy(ot[:], po_[:])
                        nc.sync.dma_start(
                            out[mb * P:(mb + 1) * P, oc * 512:(oc + 1) * 512], ot[:])
                return emit, 2 * 4 * 512 * _MM

            proj_q = []

            def pull(deficit):
                # drain due PE work: proj backlog first, then feeder (capped)
                while proj_q and deficit >= proj_q[0][1]:
                    e, c = proj_q.pop(0)
                    e()
                    deficit -= c
                return drain_feeder(ci + 1 + 7 / 8, deficit)

            # ---- fused attention loop ----
            for ci in range(nCh):
                jfull = 4 * ci
                jmax = jfull + 4
                deficit = 0.0
                for hp in range(0, HPC, 2):
                    drain_feeder(ci + hp / 8)     # hard deps for this pair
                    # pro-rata quota: spread units due by the NEXT pair across
                    # this pair's iterations instead of dumping at its start
                    d_next = ci + (hp + 2) / 8
                    base = fpos[0]
                    m = 0
                    while base + m < len(feeder) and feeder[base + m][0] <= d_next:
                        m += 1
                    n_it = 2 * ci + 4
                    it = [0]

                    def quota_drain():
                        it[0] += 1
                        want = base + min(m, -(-m * it[0] // n_it))
                        while fpos[0] < want:
                            feeder[fpos[0]][1]()
                            fpos[0] += 1
                    hs = (hp, hp + 1)
                    st = {h: dict(po=(h % 2) * HD, qc=(h // 2), vc=h * (HD + 1),
                                  py=psY.tile([HD + 1, 512], F32, tag="psY",
                                              name=f"py{ci}_{h}"))
                          for h in hs}

                    def st_mm(h, dst, jb, a):
                        s = st[h]
                        nc.tensor.matmul(
                            dst,
                            k_sb[s["po"]:s["po"] + HD,
                                 s["qc"] * T + jb * P: s["qc"] * T + (jb + 1) * P],
                            q_t[ci][s["po"]:s["po"] + HD,
                                    s["qc"] * 512 + a: (s["qc"] + 1) * 512],
                            start=True, stop=True)

                    def av_mm(h, jb, at_ap, a):
                        s = st[h]
                        nc.tensor.matmul(
                            s["py"][:, a:512],
                            v_sb[:, jb * HPC * (HD + 1) + s["vc"]:
                                 jb * HPC * (HD + 1) + s["vc"] + HD + 1],
                            at_ap,
                            start=(jb == 0), stop=(jb == jmax - 1))

                    for j0 in range(0, jfull, 2):      # full blocks, paired
                        ats = {}
                        for h in hs:
                            psp = psS.tile([P, 1024], F32, tag="psS",
                                           name=f"psp{ci}_{h}_{j0}")
                            st_mm(h, psp[:, 0:512], j0, 0)
                            st_mm(h, psp[:, 512:1024], j0 + 1, 0)
                            at = wrk.tile([P, 1024], ADT, tag="at",
                                          name=f"at{ci}_{h}_{j0}")
                            nc.scalar.activation(at[:], psp[:], Exp, scale=SCALE)
                            ats[h] = at
                        deficit += 2 * (1024 * _ACT + _ACTF) - 8 * 512 * _MM
                        deficit = pull(deficit)
                        quota_drain()
                        for h in hs:
                            av_mm(h, j0, ats[h][:, 0:512], 0)
                            av_mm(h, j0 + 1, ats[h][:, 512:1024], 0)

                    for p_ in range(4):                # crossing blocks, 2 heads packed
                        jb = jfull + p_
                        a = min(128 * p_, 256)
                        w = 512 - a
                        mw = 128 * p_ - a + P          # 128, or 256 when clamped
                        psp = psS.tile([P, 1024], F32, tag="psS",
                                       name=f"psx{ci}_{hp}_{p_}")
                        st_mm(hs[0], psp[:, a:512], jb, a)
                        st_mm(hs[1], psp[:, 512:512 + w], jb, a)
                        at = wrk.tile([P, 1024], ADT, tag="at",
                                      name=f"atx{ci}_{hp}_{p_}")
                        nc.scalar.activation(at[:, a:512 + w], psp[:, a:512 + w],
                                             Exp, scale=SCALE)
                        nc.vector.tensor_mul(
                            at[:, a:a + mw], at[:, a:a + mw],
                            mask_sb[:, 2 * P - mw:2 * P])
                        nc.vector.tensor_mul(
                            at[:, 512:512 + mw], at[:, 512:512 + mw],
                            mask_sb[:, 2 * P - mw:2 * P])
                        deficit += ((512 + w - a) * _ACT + _ACTF) - 4 * w * _MM
                        deficit = pull(deficit)
                        quota_drain()
                        av_mm(hs[0], jb, at[:, a:512], a)
                        av_mm(hs[1], jb, at[:, 512:512 + w], a)

                    # evacuate Yaug to SBUF first (frees the PSUM bank after a
                    # single copy), then normalize off the critical path
                    yas = {}
                    for h in hs:
                        ya = wrk.tile([HD + 1, 512], F32, tag="ya", bufs=4,
                                      name=f"ya{ci}_{h}")
                        nc.vector.tensor_copy(ya[:], st[h]["py"][:])
                        yas[h] = ya
                    for h in hs:
                        s = st[h]
                        ya = yas[h]
                        rt = fin.tile([1, 512], F32, tag="rt", bufs=2)
                        nc.vector.reciprocal(rt[:], ya[HD:HD + 1, :])
                        rb = fin.tile([HD, 512], F32, tag="rb", bufs=2)
                        nc.gpsimd.partition_broadcast(rb[:], rt[:])
                        nc.vector.tensor_mul(
                            yt_t[ci][s["po"]:s["po"] + HD,
                                     s["qc"] * 512:(s["qc"] + 1) * 512],
                            ya[0:HD, :], rb[:])

                # queue this chunk's proj; drain leftovers of previous chunk now
                for e, c_ in proj_q:
                    e()
                proj_q = [proj_unit(ci, mb) for mb in range(4 * ci, 4 * ci + 4)]

            drain_feeder(float(nCh))
            for e, _c in proj_q:
                e()
    return nc


_CACHE = {}


def get_nc(T=2048):
    if T not in _CACHE:
        nc = build(T)
        nc.compile()
        _CACHE[T] = nc
    return _CACHE[T]


def make_in_maps(x, W_attn, W_proj):
    Bx, T, Cx = x.shape
    Wq, Wk, Wv = W_attn[:Cx], W_attn[Cx:2 * Cx], W_attn[2 * Cx:]
    import ml_dtypes
    cva = lambda a: np.ascontiguousarray(a).astype(ml_dtypes.bfloat16)
    cv = cva if AGGR else (lambda a: np.ascontiguousarray(a))
    cvw = cva if AGGR else (lambda a: np.ascontiguousarray(a))
    cvp = cva if AGGR else (lambda a: np.ascontiguousarray(a))
    cvo = cva
    r = np.arange(P)
    tri = (r[:, None] <= r[None, :]).astype(np.float32)
    mask = np.concatenate([np.zeros((P, P), np.float32), tri], axis=1)
    ones = np.ones((P, (T // P) * HPC), np.float32)
    in_maps = []
    for core in range(8):
        b, g = divmod(core, 2)
        rows = slice(g * GQ, (g + 1) * GQ)
        in_maps.append({
            "xT": cv(x[b].T),
            "wqkT": cvw(np.concatenate([Wq[rows], Wk[rows]], 0).T),
            "wvT": cvw(Wv[rows].T),
            "wpT": cvp(W_proj[:, rows].T),
            "maskT": cva(mask),
            "onesT": cvo(ones),
        })
    return in_maps


def kernel(x, W_attn, W_proj):
    x = np.asarray(x, dtype=np.float32)
    W_attn = np.asarray(W_attn, dtype=np.float32)
    W_proj = np.asarray(W_proj, dtype=np.float32)
    Bx, T, Cx = x.shape
    assert (Bx, Cx) == (B, C) and W_attn.shape == (3 * C, C) and W_proj.shape == (C, C)
    nc = get_nc(T)
    res = run_bass_kernel_spmd(nc, make_in_maps(x, W_attn, W_proj), list(range(8)))
    out = np.empty((Bx, T, Cx), np.float32)
    for b in range(Bx):
        out[b] = res.results[2 * b]["out"] + res.results[2 * b + 1]["out"]
    return out


if __name__ == "__main__":
    rng = np.random.default_rng(0)
    x = rng.standard_normal((B, 2048, C), dtype=np.float32)
    W_attn = rng.standard_normal((3 * C, C), dtype=np.float32) * (1.0 / np.sqrt(C))
    W_proj = rng.standard_normal((C, C), dtype=np.float32) * (1.0 / np.sqrt(C))
    out = kernel(x, W_attn, W_proj)
    print("out", out.shape, out.dtype, np.abs(out).max())
